# revision 59
# baseline (speedup 1.0000x reference)
"""Causal multi-head attention on 8 TRN2 NeuronCores.

Problem: B=4, T=2048, d_model=1024, 16 heads x 64. out = softmax(causal(QK^T)/8) V Wo.

Sharding (tensor-parallel heads x data-parallel batch):
  core c -> batch b = c//2, head group g = c%2 (8 heads each).
  Each core computes a partial output  z_g[b] @ Wo[g] : [2048, 1024];
  host sums the two head-group partials per batch.

Per-core kernel (all matmul inputs bf16 = full PE rate, psum fp32):
  - host passes x[b]^T (d_model on the SBUF partition dim everywhere)
  - fused single pass over 4 query chunks of 512:
      proj (Q^T,K^T,V for the chunk) -> causal attention over k-blocks of
      128.  Per k-block one merged psum tile holds BOTH heads of a pair
      (tile_position row-split pair -> the two 64-contraction matmuls
      co-issue on disjoint PE row halves); diagonal blocks get the causal
      mask folded in as a -1e9 strictly-lower-tri PE accumulate (identity
      stationary) so the single merged exp on ACT emits exact zeros;
      V-augmented-with-ones matmuls accumulate z^T and the softmax
      denominator; divide = ACT ln/exp reciprocal + co-issued K=1
      broadcast matmul pair -> output projection -> split DMA out.
  - startup is DMA-bandwidth bound: critical loads (x chunk 0, W_Q, W_K)
    are dispatched first from three engines; W_V/W_O/constants deferred.
    Chunk-0 head-pair-0 attention interleaves into the projections.
"""
import numpy as np

import concourse.bass as bass
import concourse.tile as tile
import concourse.mybir as mybir
from concourse.vector_clock import ScopedClock
from concourse.bass_utils import run_bass_kernel_spmd

D_MODEL = 1024
D_HEAD = 64
B = 4
T = 2048
H = 8              # heads per core
HG = H * D_HEAD    # 512 head-dim columns per core
TCH = 512          # q/t chunk
NCH = T // TCH     # 4
NDM = D_MODEL // 128  # 8 d_model chunks

F32R = mybir.dt.float32r
F32 = mybir.dt.float32
BF16 = mybir.dt.bfloat16
F8 = mybir.dt.float8e4
AF = mybir.ActivationFunctionType
DR = mybir.MatmulPerfMode.DoubleRow


class _TC(tile.TileContext):
    """TileContext whose tail drain carries no sem waits (this walrus build
    rejects >1 sync wait per instruction and any wait on a Drain)."""

    def _drain_and_barrier(self, tick_clock, wait_clock):
        drain_inst = self.nc.sync.drain()
        wait_clock.add_sem_waits(
            drain_inst.ins, ScopedClock({None: tick_clock.global_clock})
        )
        si = drain_inst.ins.sync_info
        waits = list(si.on_wait) if si is not None else []
        if waits:
            drain_inst.ins.sync_info = mybir.SyncInfo(
                on_wait=[], on_update=list(si.on_update)
            )
            for w in waits:
                nop = self.nc.sync.nop(nofuse=True)
                nop.ins.sync_info = mybir.SyncInfo(on_wait=[w], on_update=[])
        self.nc.all_engine_barrier()
        popped = self.nc._tile_sem_poison_stack.pop()
        assert popped is self._sem_poison
        self.nc.clear_and_free_semaphores(list(self.sems.allocated().values()))
        self.nc.all_engine_barrier()


def _split_multi_waits(nc):
    """Move all-but-one sem wait of every instruction onto same-engine NOPs."""
    cnt = 0
    for f in nc.m.functions:
        for b in f.blocks:
            new = []
            for inst in b.instructions:
                si = inst.sync_info
                if si is not None and si.on_wait is not None:
                    waits = list(si.on_wait)
                    max_keep = 0 if inst.opcode == "Drain" else 1
                    if len(waits) > max_keep:
                        keep = waits[len(waits) - max_keep:] if max_keep else []
                        spill = waits[: len(waits) - max_keep]
                        for w in spill:
                            nop = mybir.InstNoOp(
                                name=f"I-wsplit-{cnt}", engine=inst.engine,
                                ins=[], outs=[],
                            )
                            nop.sync_info = mybir.SyncInfo(
                                on_wait=[w], on_update=[]
                            )
                            new.append(nop)
                            cnt += 1
                        inst.sync_info = mybir.SyncInfo(
                            on_wait=keep, on_update=list(si.on_update)
                        )
                new.append(inst)
            b.instructions = new
    return cnt


def _build():
    nc = bass.Bass("TRN2", target_bir_lowering=False)
    xT = nc.dram_tensor("xT", (D_MODEL, T), BF16, kind="ExternalInput")
    wq = nc.dram_tensor("wq", (D_MODEL, HG), BF16, kind="ExternalInput")
    wk = nc.dram_tensor("wk", (D_MODEL, HG), BF16, kind="ExternalInput")
    wv = nc.dram_tensor("wv", (D_MODEL, HG), BF16, kind="ExternalInput")
    wo = nc.dram_tensor("wo", (HG, D_MODEL), BF16, kind="ExternalInput")
    ident = nc.dram_tensor("ident", (128, 128), BF16, kind="ExternalInput")
    ntri = nc.dram_tensor("ntri", (128, 128), BF16, kind="ExternalInput")
    ones1 = nc.dram_tensor("ones1", (33, 64), BF16, kind="ExternalInput")
    vones = nc.dram_tensor("vones", (128, T // 128, H, 1), BF16,
                           kind="ExternalInput")
    out = nc.dram_tensor("out", (T, D_MODEL), F32, kind="ExternalOutput")

    from contextlib import ExitStack
    with _TC(nc) as tc, ExitStack() as ctx:
        consts = ctx.enter_context(tc.tile_pool(name="consts", bufs=1))
        xs_pool = ctx.enter_context(tc.tile_pool(name="xs", bufs=3))
        kt_pool = ctx.enter_context(tc.tile_pool(name="kt", bufs=1))
        v_pool = ctx.enter_context(tc.tile_pool(name="v", bufs=1))
        qt_pool = ctx.enter_context(tc.tile_pool(name="qt", bufs=3))
        zt_pool = ctx.enter_context(tc.tile_pool(name="zt", bufs=4))
        et_pool = ctx.enter_context(tc.tile_pool(name="et", bufs=8))
        sm_pool = ctx.enter_context(tc.tile_pool(name="sm", bufs=4))
        ou_pool = ctx.enter_context(tc.tile_pool(name="ou", bufs=3))
        ps_s = ctx.enter_context(tc.tile_pool(name="ps_s", bufs=2, space="PSUM"))
        ps_u = ctx.enter_context(tc.tile_pool(name="ps_u", bufs=2, space="PSUM"))
        ps_w = ctx.enter_context(tc.tile_pool(name="ps_w", bufs=2, space="PSUM"))

        xT_r = xT.ap().rearrange("(c p) t -> p c t", p=128)

        # resident weights / constants.  Startup is DMA-bandwidth bound:
        # per-chunk DMAs (each sprays its descriptors over all 16 engines)
        # dispatched from three engines in parallel, critical data first
        # (xs0+wq+wk feed the first projections); wv/wo/consts are deferred
        # so they don't steal bandwidth from the critical path.
        wq_sb = consts.tile([128, NDM, HG], BF16)
        xs0 = xs_pool.tile([128, NDM, TCH], BF16, name="xs", tag="xs")
        wk_sb = consts.tile([128, NDM, HG], BF16)
        wv_sb = consts.tile([128, NDM, HG], BF16)
        wq_r = wq.ap().rearrange("(c p) n -> p c n", p=128)
        wk_r = wk.ap().rearrange("(c p) n -> p c n", p=128)
        wv_r = wv.ap().rearrange("(c p) n -> p c n", p=128)
        for c0, cw in [(0, 1), (1, 1), (2, 2), (4, 2), (6, 2)]:
            nc.sync.dma_start(out=xs0[:, c0:c0 + cw, :],
                              in_=xT_r[:, c0:c0 + cw, 0:TCH])
            nc.scalar.dma_start(out=wq_sb[:, c0:c0 + cw, :],
                                in_=wq_r[:, c0:c0 + cw, :])
            nc.gpsimd.dma_start(out=wk_sb[:, c0:c0 + cw, :],
                                in_=wk_r[:, c0:c0 + cw, :])
        for c in range(0, NDM, 2):
            nc.gpsimd.dma_start(out=wv_sb[:, c:c + 2, :], in_=wv_r[:, c:c + 2, :])
        id_sb = consts.tile([128, 128], BF16)
        ntri_sb = consts.tile([128, 128], BF16)
        ones_sb = consts.tile([33, 64], BF16)
        wo_sb = consts.tile([128, HG // 128, D_MODEL], BF16)
        # per-chunk K^T tiles [pair-packed 128, pair, t-in-chunk] and V tiles
        # (V has a ones column so row 64 of U accumulates the denominator)
        kt_tiles = [kt_pool.tile([128, 4, TCH], BF16, name=f"kt{i}", tag=f"kt{i}")
                    for i in range(NCH)]
        v_tiles = [v_pool.tile([128, 4, H, D_HEAD + 1], BF16, name=f"v{i}",
                               tag=f"v{i}") for i in range(NCH)]
        vo_r = vones.ap().rearrange("p (a b) h o -> p a b h o", b=4)

        def late_dmas():
            nc.gpsimd.dma_start(out=id_sb, in_=ident.ap())
            nc.gpsimd.dma_start(out=ntri_sb, in_=ntri.ap())
            nc.gpsimd.dma_start(out=ones_sb, in_=ones1.ap())
            for i in range(NCH):
                nc.gpsimd.dma_start(out=v_tiles[i][:, :, :, D_HEAD:],
                                    in_=vo_r[:, i])
            # W_O is not needed until the first out-projection (~200us in),
            # but an ungated dispatch would make its 1MB transfer steal
            # startup bandwidth from the critical x0/W_Q/W_K loads.  Gate it
            # behind kt0 (written ~20us in) via a tiny dependency-carrying
            # DMA that blocks the gpsimd dispatch queue until then.
            gsc = sm_pool.tile([1, 2], BF16, name="gsc")
            nc.gpsimd.dma_start(out=gsc, in_=kt_tiles[0][0:1, 0, 0:2])
            nc.gpsimd.dma_start(
                out=wo_sb, in_=wo.ap().rearrange("(c p) n -> p c n", p=128))
        late_dmas()
        # dependency-free dummy activation: hoists the ~1.3us
        # ACT_TABLE_LOAD off the first real exp's critical path (junk in,
        # junk out; the Exp/Ln table is shared).
        atl_a = sm_pool.tile([1, 128], F32, name="atl_a")
        atl_b = sm_pool.tile([1, 128], F32, name="atl_b")
        nc.vector.memset(atl_a, 0.0)
        nc.scalar.activation(out=atl_b, in_=atl_a, func=AF.Exp, scale=0.125)

        def proj_units(ch, xs, qt_sb):
            units = []
            for dqc in range(4):
                def uq(dqc=dqc):
                    pq = ps_w.tile([128, TCH], F32, tag="ps_w", name="pq")
                    for c in range(NDM):
                        nc.tensor.matmul(
                            pq, lhsT=wq_sb[:, c, dqc * 128:(dqc + 1) * 128],
                            rhs=xs[:, c, :], start=(c == 0), stop=(c == NDM - 1))
                    nc.vector.tensor_copy(out=qt_sb[:, dqc, :], in_=pq)
                units.append(uq)
                def uk(dqc=dqc):
                    pk = ps_w.tile([128, TCH], F32, tag="ps_w", name="pk")
                    for c in range(NDM):
                        nc.tensor.matmul(
                            pk, lhsT=wk_sb[:, c, dqc * 128:(dqc + 1) * 128],
                            rhs=xs[:, c, :], start=(c == 0), stop=(c == NDM - 1))
                    nc.vector.tensor_copy(out=kt_tiles[ch][:, dqc, :], in_=pk)
                units.append(uk)
            for tt in range(4):
                def uv(tt=tt):
                    pv = ps_w.tile([128, HG], F32, tag="ps_w", name="pv")
                    for c in range(NDM):
                        nc.tensor.matmul(
                            pv, lhsT=xs[:, c, tt * 128:(tt + 1) * 128],
                            rhs=wv_sb[:, c, :], start=(c == 0), stop=(c == NDM - 1))
                    nc.vector.tensor_copy(
                        out=v_tiles[ch][:, tt, :, 0:D_HEAD],
                        in_=pv.rearrange("p (h d) -> p h d", h=H))
                units.append(uv)
            return units

        def outproj_units(ch, zt_sb):
            units = []
            q0 = ch * TCH
            for tt in range(4):
                def uo(tt=tt):
                    o_sb = ou_pool.tile([128, D_MODEL], F32, name="o_sb")
                    r0 = q0 + tt * 128
                    for dc in range(2):
                        po = ps_w.tile([128, 512], F32, tag="ps_w", name="po")
                        for kc in range(4):
                            nc.tensor.matmul(
                                po, lhsT=zt_sb[:, kc, tt * 128:(tt + 1) * 128],
                                rhs=wo_sb[:, kc, dc * 512:(dc + 1) * 512],
                                start=(kc == 0), stop=(kc == 3))
                        nc.vector.tensor_copy(
                            out=o_sb[:, dc * 512:(dc + 1) * 512], in_=po)
                        nc.sync.dma_start(
                            out=out.ap()[r0:r0 + 128, dc * 512:(dc + 1) * 512],
                            in_=o_sb[:, dc * 512:(dc + 1) * 512])
                units.append(uo)
            return units

        def attention_units(ch, qt_sb, zt_sb):
            """Units: per head-pair, per k-block: a score pair (one merged
            psum tile, tile_position row-split -> co-issue), causal mask
            folded in as a -1e9 PE accumulate, one merged exp straight to
            fp8; per k-block PAIR one fp8 DoubleRow AV per head (contracts
            both k-blocks at 2 values/partition = half the PE rows); then
            the division."""
            units = []
            nkb = 4 * ch + 4
            state = {}
            for hp in range(4):
                def u_alloc(hp=hp):
                    state[hp] = [ps_u.tile([D_HEAD + 1, TCH], F32, name="u_ps",
                                           tag="u_ps") for _ in range(2)]
                units.append(u_alloc)
                for kb in range(nkb):
                    def u_kb(hp=hp, kb=kb):
                        u_ps = state[hp]
                        j = kb - 4 * ch
                        ca = 128 * j if j > 0 else 0
                        kt_t = kt_tiles[kb // 4]
                        off = (kb % 4) * 128
                        sp = ps_s.tile([128, 2, TCH], F32, name="sp", tag="s2")
                        diag = j >= 0
                        for par in range(2):
                            p0 = 64 * par
                            nc.tensor.matmul(
                                sp[:, par, ca:],
                                lhsT=kt_t[p0:p0 + 64, hp, off:off + 128],
                                rhs=qt_sb[p0:p0 + 64, hp, ca:],
                                start=True, stop=not diag,
                                tile_position=(p0, 0))
                        if diag:
                            for par in range(2):
                                nc.tensor.matmul(
                                    sp[:, par, ca:ca + 128],
                                    lhsT=id_sb, rhs=ntri_sb,
                                    start=False, stop=True)
                        et = et_pool.tile([128, 2, TCH], BF16, name="et",
                                          tag="et")
                        spf = sp.rearrange("p a b -> p (a b)")
                        etf = et.rearrange("p a b -> p (a b)")
                        if j >= 2:
                            # deep diagonal: the merged range would exp the
                            # par1 garbage gap [TCH:TCH+ca); two exps are
                            # cheaper and release par0's AV earlier.
                            nc.scalar.activation(
                                out=etf[:, ca:TCH], in_=spf[:, ca:TCH],
                                func=AF.Exp, scale=0.125)
                            nc.scalar.activation(
                                out=etf[:, TCH + ca:], in_=spf[:, TCH + ca:],
                                func=AF.Exp, scale=0.125)
                        else:
                            nc.scalar.activation(
                                out=etf[:, ca:], in_=spf[:, ca:],
                                func=AF.Exp, scale=0.125)
                        for par in range(2):
                            h = 2 * hp + par
                            nc.tensor.matmul(
                                u_ps[par][:, ca:],
                                lhsT=v_tiles[kb // 4][:, kb % 4, h, :],
                                rhs=et[:, par, ca:],
                                start=(kb == 0), stop=(kb == nkb - 1))
                    units.append(u_kb)
                def u_div(hp=hp):
                    # zt = U[0:64] / D (D = U row 64): 1/D via DVE
                    # reciprocal; broadcast over 64 partitions with a pair of
                    # K=1 matmuls co-issued at PE row positions 0 and 32
                    # (rcp rows live at partitions 0 and 32 to feed them).
                    u_ps = state[hp]
                    # 1/D = exp(-ln D) on ACT (DVE reciprocal is iterative
                    # ~3.4us; custom DVE approx ops fail codegen; ACT
                    # Reciprocal swaps tables).  Both pars' ln land in one
                    # tile (rows 0/32) and a SINGLE exp covers both, so the
                    # two K=1 broadcast matmuls share one release event and
                    # co-issue at PE row positions 0/32.  Rows 1-31 hold
                    # junk; only rows 0/32 are ever read downstream.
                    lnd = sm_pool.tile([33, TCH], F32, name="lnd")
                    for par in range(2):
                        nc.scalar.activation(
                            out=lnd[32 * par:32 * par + 1, :],
                            in_=u_ps[par][D_HEAD:D_HEAD + 1, :],
                            func=AF.Ln)
                    rcp = sm_pool.tile([33, TCH], BF16, name="rcp")
                    nc.scalar.activation(out=rcp, in_=lnd,
                                         func=AF.Exp, scale=-1.0)
                    db_ps = ps_s.tile([64, 2, TCH], F32, tag="s2", name="db_ps")
                    for par in range(2):
                        nc.tensor.matmul(
                            db_ps[:, par, :],
                            lhsT=ones_sb[32 * par:32 * par + 1, :],
                            rhs=rcp[32 * par:32 * par + 1, :],
                            start=True, stop=True,
                            tile_position=(32 * par, 0))
                    rb = sm_pool.tile([64, 2, TCH], F32, name="rb")
                    nc.vector.tensor_copy(out=rb, in_=db_ps)
                    for par in range(2):
                        nc.vector.tensor_mul(
                            zt_sb[64 * par:64 * par + 64, hp, :],
                            u_ps[par][0:D_HEAD, :], rb[:, par, :])
                u_div.is_div = True
                units.append(u_div)
            return units

        # ---- software-pipelined emission ----
        # chunk 0 projections up front; then for each chunk, its attention
        # units interleaved with (prev chunk's out-proj + next chunk's proj).
        qt_tiles = [None] * NCH
        xs_tiles = [xs0] + [None] * (NCH - 1)
        zt_tiles = [None] * NCH

        def stage_proj(ch):
            if ch >= NCH:
                return []
            if ch > 0:
                xs_tiles[ch] = xs_pool.tile([128, NDM, TCH], BF16, name="xs",
                                            tag="xs")
                # gate this 1MB load behind the previous chunk's first K^T
                # write so its dispatch (otherwise dependency-free, ~4us in)
                # does not steal startup DMA bandwidth; it still lands well
                # before this chunk's projections need it.
                gsx = sm_pool.tile([1, 2], BF16, name="gsx")
                nc.sync.dma_start(out=gsx, in_=kt_tiles[ch - 1][0:1, 0, 0:2])
                nc.sync.dma_start(
                    out=xs_tiles[ch],
                    in_=xT_r[:, :, ch * TCH:(ch + 1) * TCH])
            qt_tiles[ch] = qt_pool.tile([128, 4, TCH], BF16, name="qt",
                                        tag="qt")
            return proj_units(ch, xs_tiles[ch], qt_tiles[ch])

        # chunk-0 prologue: interleave head-pair 0's attention into proj0 as
        # its inputs become ready (uq0/uk_j/uv_j), so ACT exps start early
        # and the DMA-paced startup has PE work to chew on.
        P0 = stage_proj(0)  # [uq0,uk0, uq1,uk1, uq2,uk2, uq3,uk3, uv0..uv3]
        zt_tiles[0] = zt_pool.tile([128, 4, TCH], BF16, name="zt", tag="zt")
        A0 = attention_units(0, qt_tiles[0], zt_tiles[0])
        for u in [P0[0], P0[1], P0[8], A0[0], A0[1],
                  P0[3], P0[9], A0[2],
                  P0[5], P0[10], A0[3],
                  P0[7], P0[11], A0[4], A0[5]]:
            u()
        # fill schedule: att0 | proj1, att1 | proj2, att2 | proj3+outproj0,
        # att3 | outproj1+outproj2, then outproj3.  Later chunks have more
        # attention (ACT-paced) so they get the spare PE fill.
        for ch in range(NCH):
            if ch == 0:
                au = ([P0[2]] + A0[6:12] + [P0[4]] + A0[12:18]
                      + [P0[6]] + A0[18:24])
            else:
                zt_tiles[ch] = zt_pool.tile([128, 4, TCH], BF16, name="zt",
                                            tag="zt")
                au = attention_units(ch, qt_tiles[ch], zt_tiles[ch])
            fill = []
            if ch < NCH - 1:
                fill += stage_proj(ch + 1)
            if ch == 3:
                fill += outproj_units(0, zt_tiles[0])
                fill += outproj_units(1, zt_tiles[1])
                fill += outproj_units(2, zt_tiles[2])
            k = 0
            for i, a in enumerate(au):
                if getattr(a, "is_div", False):
                    # pull fill ahead of the divide: its broadcast matmul
                    # waits ~1.3us for the ACT reciprocal and the PE issues
                    # in order, so anything behind it would stall too.
                    want = min(len(fill), k + 3)
                    while k < want:
                        fill[k]()
                        k += 1
                a()
                want = (i + 1) * len(fill) // len(au)
                while k < want:
                    fill[k]()
                    k += 1
            while k < len(fill):
                fill[k]()
                k += 1
        for u in outproj_units(NCH - 1, zt_tiles[NCH - 1]):
            u()


    _split_multi_waits(nc)
    return nc


_NC_CACHE = None


def _get_nc():
    global _NC_CACHE
    if _NC_CACHE is None:
        _NC_CACHE = _build()
    return _NC_CACHE


def _make_in_maps(x, W_Q, W_K, W_V, W_O):
    import ml_dtypes
    bf16 = ml_dtypes.bfloat16
    x = np.asarray(x, dtype=np.float32)
    W_Q = np.asarray(W_Q, dtype=np.float32).astype(bf16)
    W_K = np.asarray(W_K, dtype=np.float32).astype(bf16)
    W_V = np.asarray(W_V, dtype=np.float32).astype(bf16)
    W_O = np.asarray(W_O, dtype=np.float32).astype(bf16)

    f8 = ml_dtypes.float8_e4m3
    ident = np.eye(128, dtype=np.float32).astype(bf16)
    # mask[k, q] = -1e9 where q < k (strictly below diagonal), else 0
    ntri = np.tril(np.full((128, 128), -1e9, dtype=np.float32), k=-1).astype(bf16)
    ones1 = np.ones((33, 64), dtype=bf16)
    vones = np.ones((128, T // 128, H, 1), dtype=bf16)

    in_maps = []
    for core in range(8):
        b, g = core // 2, core % 2
        cs = slice(g * HG, (g + 1) * HG)
        in_maps.append({
            "xT": np.ascontiguousarray(x[b].T).astype(bf16),
            "wq": np.ascontiguousarray(W_Q[:, cs]),
            "wk": np.ascontiguousarray(W_K[:, cs]),
            "wv": np.ascontiguousarray(W_V[:, cs]),
            "wo": np.ascontiguousarray(W_O[cs, :]),
            "ident": ident, "ntri": ntri, "ones1": ones1, "vones": vones,
        })
    return in_maps


def kernel(x, W_Q, W_K, W_V, W_O):
    in_maps = _make_in_maps(x, W_Q, W_K, W_V, W_O)
    nc = _get_nc()
    res = run_bass_kernel_spmd(nc, in_maps, core_ids=list(range(8)))
    outs = [res.results[c]["out"] for c in range(8)]
    full = np.stack([outs[2 * b] + outs[2 * b + 1] for b in range(B)], axis=0)
    return full


# revision 61
# speedup vs baseline: 1.0036x; 1.0036x over previous
"""Causal multi-head attention on 8 TRN2 NeuronCores.

Problem: B=4, T=2048, d_model=1024, 16 heads x 64. out = softmax(causal(QK^T)/8) V Wo.

Sharding (tensor-parallel heads x data-parallel batch):
  core c -> batch b = c//2, head group g = c%2 (8 heads each).
  Each core computes a partial output  z_g[b] @ Wo[g] : [2048, 1024];
  host sums the two head-group partials per batch.

Per-core kernel (all matmul inputs bf16 = full PE rate, psum fp32):
  - host passes x[b]^T (d_model on the SBUF partition dim everywhere)
  - fused single pass over 4 query chunks of 512:
      proj (Q^T,K^T,V for the chunk) -> causal attention over k-blocks of
      128.  Per k-block one merged psum tile holds BOTH heads of a pair
      (tile_position row-split pair -> the two 64-contraction matmuls
      co-issue on disjoint PE row halves); diagonal blocks get the causal
      mask folded in as a -1e9 strictly-lower-tri PE accumulate (identity
      stationary) so the single merged exp on ACT emits exact zeros;
      V-augmented-with-ones matmuls accumulate z^T and the softmax
      denominator; divide = ACT ln/exp reciprocal + co-issued K=1
      broadcast matmul pair -> output projection -> split DMA out.
  - startup is DMA-bandwidth bound: critical loads (x chunk 0, W_Q, W_K)
    are dispatched first from three engines; W_V/W_O/constants deferred.
    Chunk-0 head-pair-0 attention interleaves into the projections.
"""
import numpy as np

import concourse.bass as bass
import concourse.tile as tile
import concourse.mybir as mybir
from concourse.vector_clock import ScopedClock
from concourse.bass_utils import run_bass_kernel_spmd

D_MODEL = 1024
D_HEAD = 64
B = 4
T = 2048
H = 8              # heads per core
HG = H * D_HEAD    # 512 head-dim columns per core
TCH = 512          # q/t chunk
NCH = T // TCH     # 4
NDM = D_MODEL // 128  # 8 d_model chunks

F32R = mybir.dt.float32r
F32 = mybir.dt.float32
BF16 = mybir.dt.bfloat16
F8 = mybir.dt.float8e4
AF = mybir.ActivationFunctionType
DR = mybir.MatmulPerfMode.DoubleRow


class _TC(tile.TileContext):
    """TileContext whose tail drain carries no sem waits (this walrus build
    rejects >1 sync wait per instruction and any wait on a Drain)."""

    def _drain_and_barrier(self, tick_clock, wait_clock):
        drain_inst = self.nc.sync.drain()
        wait_clock.add_sem_waits(
            drain_inst.ins, ScopedClock({None: tick_clock.global_clock})
        )
        si = drain_inst.ins.sync_info
        waits = list(si.on_wait) if si is not None else []
        if waits:
            drain_inst.ins.sync_info = mybir.SyncInfo(
                on_wait=[], on_update=list(si.on_update)
            )
            for w in waits:
                nop = self.nc.sync.nop(nofuse=True)
                nop.ins.sync_info = mybir.SyncInfo(on_wait=[w], on_update=[])
        self.nc.all_engine_barrier()
        popped = self.nc._tile_sem_poison_stack.pop()
        assert popped is self._sem_poison
        self.nc.clear_and_free_semaphores(list(self.sems.allocated().values()))
        self.nc.all_engine_barrier()


def _split_multi_waits(nc):
    """Move all-but-one sem wait of every instruction onto same-engine NOPs."""
    cnt = 0
    for f in nc.m.functions:
        for b in f.blocks:
            new = []
            for inst in b.instructions:
                si = inst.sync_info
                if si is not None and si.on_wait is not None:
                    waits = list(si.on_wait)
                    max_keep = 0 if inst.opcode == "Drain" else 1
                    if len(waits) > max_keep:
                        keep = waits[len(waits) - max_keep:] if max_keep else []
                        spill = waits[: len(waits) - max_keep]
                        for w in spill:
                            nop = mybir.InstNoOp(
                                name=f"I-wsplit-{cnt}", engine=inst.engine,
                                ins=[], outs=[],
                            )
                            nop.sync_info = mybir.SyncInfo(
                                on_wait=[w], on_update=[]
                            )
                            new.append(nop)
                            cnt += 1
                        inst.sync_info = mybir.SyncInfo(
                            on_wait=keep, on_update=list(si.on_update)
                        )
                new.append(inst)
            b.instructions = new
    return cnt


def _build():
    nc = bass.Bass("TRN2", target_bir_lowering=False)
    xT = nc.dram_tensor("xT", (D_MODEL, T), BF16, kind="ExternalInput")
    wq = nc.dram_tensor("wq", (D_MODEL, HG), BF16, kind="ExternalInput")
    wk = nc.dram_tensor("wk", (D_MODEL, HG), BF16, kind="ExternalInput")
    wv = nc.dram_tensor("wv", (D_MODEL, HG), BF16, kind="ExternalInput")
    wo = nc.dram_tensor("wo", (HG, D_MODEL), BF16, kind="ExternalInput")
    ident = nc.dram_tensor("ident", (128, 128), BF16, kind="ExternalInput")
    ntri = nc.dram_tensor("ntri", (128, 128), BF16, kind="ExternalInput")
    ones1 = nc.dram_tensor("ones1", (33, 64), BF16, kind="ExternalInput")
    vones = nc.dram_tensor("vones", (128, T // 128, H, 1), BF16,
                           kind="ExternalInput")
    out = nc.dram_tensor("out", (T, D_MODEL), F32, kind="ExternalOutput")

    from contextlib import ExitStack
    with _TC(nc) as tc, ExitStack() as ctx:
        consts = ctx.enter_context(tc.tile_pool(name="consts", bufs=1))
        xs_pool = ctx.enter_context(tc.tile_pool(name="xs", bufs=3))
        kt_pool = ctx.enter_context(tc.tile_pool(name="kt", bufs=1))
        v_pool = ctx.enter_context(tc.tile_pool(name="v", bufs=1))
        qt_pool = ctx.enter_context(tc.tile_pool(name="qt", bufs=3))
        zt_pool = ctx.enter_context(tc.tile_pool(name="zt", bufs=4))
        et_pool = ctx.enter_context(tc.tile_pool(name="et", bufs=8))
        sm_pool = ctx.enter_context(tc.tile_pool(name="sm", bufs=4))
        ou_pool = ctx.enter_context(tc.tile_pool(name="ou", bufs=3))
        ps_s = ctx.enter_context(tc.tile_pool(name="ps_s", bufs=2, space="PSUM"))
        ps_u = ctx.enter_context(tc.tile_pool(name="ps_u", bufs=2, space="PSUM"))
        ps_w = ctx.enter_context(tc.tile_pool(name="ps_w", bufs=2, space="PSUM"))

        xT_r = xT.ap().rearrange("(c p) t -> p c t", p=128)

        # resident weights / constants.  Startup is DMA-bandwidth bound:
        # per-chunk DMAs (each sprays its descriptors over all 16 engines)
        # dispatched from three engines in parallel, critical data first
        # (xs0+wq+wk feed the first projections); wv/wo/consts are deferred
        # so they don't steal bandwidth from the critical path.
        wq_sb = consts.tile([128, NDM, HG], BF16)
        xs0 = xs_pool.tile([128, NDM, TCH], BF16, name="xs", tag="xs")
        wk_sb = consts.tile([128, NDM, HG], BF16)
        wv_sb = consts.tile([128, NDM, HG], BF16)
        wq_r = wq.ap().rearrange("(c p) n -> p c n", p=128)
        wk_r = wk.ap().rearrange("(c p) n -> p c n", p=128)
        wv_r = wv.ap().rearrange("(c p) n -> p c n", p=128)
        for c0, cw in [(0, 1), (1, 1), (2, 2), (4, 2), (6, 2)]:
            nc.sync.dma_start(out=xs0[:, c0:c0 + cw, :],
                              in_=xT_r[:, c0:c0 + cw, 0:TCH])
            nc.scalar.dma_start(out=wq_sb[:, c0:c0 + cw, :],
                                in_=wq_r[:, c0:c0 + cw, :])
            nc.gpsimd.dma_start(out=wk_sb[:, c0:c0 + cw, :],
                                in_=wk_r[:, c0:c0 + cw, :])
        for c in range(0, NDM, 2):
            nc.gpsimd.dma_start(out=wv_sb[:, c:c + 2, :], in_=wv_r[:, c:c + 2, :])
        id_sb = consts.tile([128, 128], BF16)
        ntri_sb = consts.tile([128, 128], BF16)
        ones_sb = consts.tile([33, 64], BF16)
        wo_sb = consts.tile([128, HG // 128, D_MODEL], BF16)
        # per-chunk K^T tiles [pair-packed 128, pair, t-in-chunk] and V tiles
        # (V has a ones column so row 64 of U accumulates the denominator)
        kt_tiles = [kt_pool.tile([128, 4, TCH], BF16, name=f"kt{i}", tag=f"kt{i}")
                    for i in range(NCH)]
        v_tiles = [v_pool.tile([128, 4, H, D_HEAD + 1], BF16, name=f"v{i}",
                               tag=f"v{i}") for i in range(NCH)]
        vo_r = vones.ap().rearrange("p (a b) h o -> p a b h o", b=4)

        def late_dmas():
            nc.gpsimd.dma_start(out=id_sb, in_=ident.ap())
            nc.gpsimd.dma_start(out=ntri_sb, in_=ntri.ap())
            nc.gpsimd.dma_start(out=ones_sb, in_=ones1.ap())
            for i in range(NCH):
                nc.gpsimd.dma_start(out=v_tiles[i][:, :, :, D_HEAD:],
                                    in_=vo_r[:, i])
            # W_O is not needed until the first out-projection (~200us in),
            # but an ungated dispatch would put its 1MB transfer in flight
            # at ~4us, stealing startup bandwidth from the critical
            # x0/W_Q/W_K loads.  Gate it behind kt0 (written ~20us in) via
            # a tiny dependency-carrying DMA that blocks the otherwise-idle
            # gpsimd dispatch queue until then.
            gsc = sm_pool.tile([1, 2], BF16, name="gsc")
            nc.gpsimd.dma_start(out=gsc, in_=kt_tiles[0][0:1, 0, 0:2])
            nc.gpsimd.dma_start(
                out=wo_sb, in_=wo.ap().rearrange("(c p) n -> p c n", p=128))
        late_dmas()
        # dependency-free dummy activation: hoists the ~1.3us
        # ACT_TABLE_LOAD off the first real exp's critical path (junk in,
        # junk out; the Exp/Ln table is shared).
        atl_a = sm_pool.tile([1, 128], F32, name="atl_a")
        atl_b = sm_pool.tile([1, 128], F32, name="atl_b")
        nc.vector.memset(atl_a, 0.0)
        nc.scalar.activation(out=atl_b, in_=atl_a, func=AF.Exp, scale=0.125)

        def proj_units(ch, xs, qt_sb):
            units = []
            for dqc in range(4):
                def uq(dqc=dqc):
                    pq = ps_w.tile([128, TCH], F32, tag="ps_w", name="pq")
                    for c in range(NDM):
                        nc.tensor.matmul(
                            pq, lhsT=wq_sb[:, c, dqc * 128:(dqc + 1) * 128],
                            rhs=xs[:, c, :], start=(c == 0), stop=(c == NDM - 1))
                    nc.vector.tensor_copy(out=qt_sb[:, dqc, :], in_=pq)
                units.append(uq)
                def uk(dqc=dqc):
                    pk = ps_w.tile([128, TCH], F32, tag="ps_w", name="pk")
                    for c in range(NDM):
                        nc.tensor.matmul(
                            pk, lhsT=wk_sb[:, c, dqc * 128:(dqc + 1) * 128],
                            rhs=xs[:, c, :], start=(c == 0), stop=(c == NDM - 1))
                    nc.vector.tensor_copy(out=kt_tiles[ch][:, dqc, :], in_=pk)
                units.append(uk)
            for tt in range(4):
                def uv(tt=tt):
                    pv = ps_w.tile([128, HG], F32, tag="ps_w", name="pv")
                    for c in range(NDM):
                        nc.tensor.matmul(
                            pv, lhsT=xs[:, c, tt * 128:(tt + 1) * 128],
                            rhs=wv_sb[:, c, :], start=(c == 0), stop=(c == NDM - 1))
                    nc.vector.tensor_copy(
                        out=v_tiles[ch][:, tt, :, 0:D_HEAD],
                        in_=pv.rearrange("p (h d) -> p h d", h=H))
                units.append(uv)
            return units

        def outproj_units(ch, zt_sb):
            units = []
            q0 = ch * TCH
            for tt in range(4):
                def uo(tt=tt):
                    o_sb = ou_pool.tile([128, D_MODEL], F32, name="o_sb")
                    r0 = q0 + tt * 128
                    for dc in range(2):
                        po = ps_w.tile([128, 512], F32, tag="ps_w", name="po")
                        for kc in range(4):
                            nc.tensor.matmul(
                                po, lhsT=zt_sb[:, kc, tt * 128:(tt + 1) * 128],
                                rhs=wo_sb[:, kc, dc * 512:(dc + 1) * 512],
                                start=(kc == 0), stop=(kc == 3))
                        nc.vector.tensor_copy(
                            out=o_sb[:, dc * 512:(dc + 1) * 512], in_=po)
                        nc.sync.dma_start(
                            out=out.ap()[r0:r0 + 128, dc * 512:(dc + 1) * 512],
                            in_=o_sb[:, dc * 512:(dc + 1) * 512])
                units.append(uo)
            return units

        def attention_units(ch, qt_sb, zt_sb):
            """Units: per head-pair, per k-block: a score pair (one merged
            psum tile, tile_position row-split -> co-issue), causal mask
            folded in as a -1e9 PE accumulate, one merged exp straight to
            fp8; per k-block PAIR one fp8 DoubleRow AV per head (contracts
            both k-blocks at 2 values/partition = half the PE rows); then
            the division."""
            units = []
            nkb = 4 * ch + 4
            state = {}
            for hp in range(4):
                def u_alloc(hp=hp):
                    state[hp] = [ps_u.tile([D_HEAD + 1, TCH], F32, name="u_ps",
                                           tag="u_ps") for _ in range(2)]
                units.append(u_alloc)
                for kb in range(nkb):
                    def u_kb(hp=hp, kb=kb):
                        u_ps = state[hp]
                        j = kb - 4 * ch
                        ca = 128 * j if j > 0 else 0
                        kt_t = kt_tiles[kb // 4]
                        off = (kb % 4) * 128
                        sp = ps_s.tile([128, 2, TCH], F32, name="sp", tag="s2")
                        diag = j >= 0
                        for par in range(2):
                            p0 = 64 * par
                            nc.tensor.matmul(
                                sp[:, par, ca:],
                                lhsT=kt_t[p0:p0 + 64, hp, off:off + 128],
                                rhs=qt_sb[p0:p0 + 64, hp, ca:],
                                start=True, stop=not diag,
                                tile_position=(p0, 0))
                        if diag:
                            for par in range(2):
                                nc.tensor.matmul(
                                    sp[:, par, ca:ca + 128],
                                    lhsT=id_sb, rhs=ntri_sb,
                                    start=False, stop=True)
                        et = et_pool.tile([128, 2, TCH], BF16, name="et",
                                          tag="et")
                        spf = sp.rearrange("p a b -> p (a b)")
                        etf = et.rearrange("p a b -> p (a b)")
                        if j >= 2:
                            # deep diagonal: the merged range would exp the
                            # par1 garbage gap [TCH:TCH+ca); two exps are
                            # cheaper and release par0's AV earlier.
                            nc.scalar.activation(
                                out=etf[:, ca:TCH], in_=spf[:, ca:TCH],
                                func=AF.Exp, scale=0.125)
                            nc.scalar.activation(
                                out=etf[:, TCH + ca:], in_=spf[:, TCH + ca:],
                                func=AF.Exp, scale=0.125)
                        else:
                            nc.scalar.activation(
                                out=etf[:, ca:], in_=spf[:, ca:],
                                func=AF.Exp, scale=0.125)
                        for par in range(2):
                            h = 2 * hp + par
                            nc.tensor.matmul(
                                u_ps[par][:, ca:],
                                lhsT=v_tiles[kb // 4][:, kb % 4, h, :],
                                rhs=et[:, par, ca:],
                                start=(kb == 0), stop=(kb == nkb - 1))
                    units.append(u_kb)
                def u_div(hp=hp):
                    # zt = U[0:64] / D (D = U row 64): 1/D via DVE
                    # reciprocal; broadcast over 64 partitions with a pair of
                    # K=1 matmuls co-issued at PE row positions 0 and 32
                    # (rcp rows live at partitions 0 and 32 to feed them).
                    u_ps = state[hp]
                    # 1/D = exp(-ln D) on ACT (DVE reciprocal is iterative
                    # ~3.4us; custom DVE approx ops fail codegen; ACT
                    # Reciprocal swaps tables).  Both pars' ln land in one
                    # tile (rows 0/32) and a SINGLE exp covers both, so the
                    # two K=1 broadcast matmuls share one release event and
                    # co-issue at PE row positions 0/32.  Rows 1-31 hold
                    # junk; only rows 0/32 are ever read downstream.
                    lnd = sm_pool.tile([33, TCH], F32, name="lnd")
                    for par in range(2):
                        nc.scalar.activation(
                            out=lnd[32 * par:32 * par + 1, :],
                            in_=u_ps[par][D_HEAD:D_HEAD + 1, :],
                            func=AF.Ln)
                    rcp = sm_pool.tile([33, TCH], BF16, name="rcp")
                    nc.scalar.activation(out=rcp, in_=lnd,
                                         func=AF.Exp, scale=-1.0)
                    db_ps = ps_s.tile([64, 2, TCH], F32, tag="s2", name="db_ps")
                    for par in range(2):
                        nc.tensor.matmul(
                            db_ps[:, par, :],
                            lhsT=ones_sb[32 * par:32 * par + 1, :],
                            rhs=rcp[32 * par:32 * par + 1, :],
                            start=True, stop=True,
                            tile_position=(32 * par, 0))
                    rb = sm_pool.tile([64, 2, TCH], F32, name="rb")
                    nc.vector.tensor_copy(out=rb, in_=db_ps)
                    for par in range(2):
                        nc.vector.tensor_mul(
                            zt_sb[64 * par:64 * par + 64, hp, :],
                            u_ps[par][0:D_HEAD, :], rb[:, par, :])
                u_div.is_div = True
                units.append(u_div)
            return units

        # ---- software-pipelined emission ----
        # chunk 0 projections up front; then for each chunk, its attention
        # units interleaved with (prev chunk's out-proj + next chunk's proj).
        qt_tiles = [None] * NCH
        xs_tiles = [xs0] + [None] * (NCH - 1)
        zt_tiles = [None] * NCH

        def stage_proj(ch):
            if ch >= NCH:
                return []
            if ch > 0:
                xs_tiles[ch] = xs_pool.tile([128, NDM, TCH], BF16, name="xs",
                                            tag="xs")
                nc.sync.dma_start(
                    out=xs_tiles[ch],
                    in_=xT_r[:, :, ch * TCH:(ch + 1) * TCH])
            qt_tiles[ch] = qt_pool.tile([128, 4, TCH], BF16, name="qt",
                                        tag="qt")
            return proj_units(ch, xs_tiles[ch], qt_tiles[ch])

        # chunk-0 prologue: interleave head-pair 0's attention into proj0 as
        # its inputs become ready (uq0/uk_j/uv_j), so ACT exps start early
        # and the DMA-paced startup has PE work to chew on.
        P0 = stage_proj(0)  # [uq0,uk0, uq1,uk1, uq2,uk2, uq3,uk3, uv0..uv3]
        zt_tiles[0] = zt_pool.tile([128, 4, TCH], BF16, name="zt", tag="zt")
        A0 = attention_units(0, qt_tiles[0], zt_tiles[0])
        for u in [P0[0], P0[1], P0[8], A0[0], A0[1],
                  P0[3], P0[9], A0[2],
                  P0[5], P0[10], A0[3],
                  P0[7], P0[11], A0[4], A0[5]]:
            u()
        # fill schedule: att0 | proj1, att1 | proj2, att2 | proj3+outproj0,
        # att3 | outproj1+outproj2, then outproj3.  Later chunks have more
        # attention (ACT-paced) so they get the spare PE fill.
        for ch in range(NCH):
            if ch == 0:
                au = ([P0[2]] + A0[6:12] + [P0[4]] + A0[12:18]
                      + [P0[6]] + A0[18:24])
            else:
                zt_tiles[ch] = zt_pool.tile([128, 4, TCH], BF16, name="zt",
                                            tag="zt")
                au = attention_units(ch, qt_tiles[ch], zt_tiles[ch])
            fill = []
            if ch < NCH - 1:
                fill += stage_proj(ch + 1)
            if ch == 3:
                fill += outproj_units(0, zt_tiles[0])
                fill += outproj_units(1, zt_tiles[1])
                fill += outproj_units(2, zt_tiles[2])
            k = 0
            for i, a in enumerate(au):
                if getattr(a, "is_div", False):
                    # pull fill ahead of the divide: its broadcast matmul
                    # waits ~1.3us for the ACT reciprocal and the PE issues
                    # in order, so anything behind it would stall too.
                    want = min(len(fill), k + 3)
                    while k < want:
                        fill[k]()
                        k += 1
                a()
                want = (i + 1) * len(fill) // len(au)
                while k < want:
                    fill[k]()
                    k += 1
            while k < len(fill):
                fill[k]()
                k += 1
        for u in outproj_units(NCH - 1, zt_tiles[NCH - 1]):
            u()


    _split_multi_waits(nc)
    return nc


_NC_CACHE = None


def _get_nc():
    global _NC_CACHE
    if _NC_CACHE is None:
        _NC_CACHE = _build()
    return _NC_CACHE


def _make_in_maps(x, W_Q, W_K, W_V, W_O):
    import ml_dtypes
    bf16 = ml_dtypes.bfloat16
    x = np.asarray(x, dtype=np.float32)
    W_Q = np.asarray(W_Q, dtype=np.float32).astype(bf16)
    W_K = np.asarray(W_K, dtype=np.float32).astype(bf16)
    W_V = np.asarray(W_V, dtype=np.float32).astype(bf16)
    W_O = np.asarray(W_O, dtype=np.float32).astype(bf16)

    f8 = ml_dtypes.float8_e4m3
    ident = np.eye(128, dtype=np.float32).astype(bf16)
    # mask[k, q] = -1e9 where q < k (strictly below diagonal), else 0
    ntri = np.tril(np.full((128, 128), -1e9, dtype=np.float32), k=-1).astype(bf16)
    ones1 = np.ones((33, 64), dtype=bf16)
    vones = np.ones((128, T // 128, H, 1), dtype=bf16)

    in_maps = []
    for core in range(8):
        b, g = core // 2, core % 2
        cs = slice(g * HG, (g + 1) * HG)
        in_maps.append({
            "xT": np.ascontiguousarray(x[b].T).astype(bf16),
            "wq": np.ascontiguousarray(W_Q[:, cs]),
            "wk": np.ascontiguousarray(W_K[:, cs]),
            "wv": np.ascontiguousarray(W_V[:, cs]),
            "wo": np.ascontiguousarray(W_O[cs, :]),
            "ident": ident, "ntri": ntri, "ones1": ones1, "vones": vones,
        })
    return in_maps


def kernel(x, W_Q, W_K, W_V, W_O):
    in_maps = _make_in_maps(x, W_Q, W_K, W_V, W_O)
    nc = _get_nc()
    res = run_bass_kernel_spmd(nc, in_maps, core_ids=list(range(8)))
    outs = [res.results[c]["out"] for c in range(8)]
    full = np.stack([outs[2 * b] + outs[2 * b + 1] for b in range(B)], axis=0)
    return full


# revision 63
# speedup vs baseline: 1.0224x; 1.0187x over previous
"""Causal multi-head attention on 8 TRN2 NeuronCores.

Problem: B=4, T=2048, d_model=1024, 16 heads x 64. out = softmax(causal(QK^T)/8) V Wo.

Sharding (tensor-parallel heads x data-parallel batch):
  core c -> batch b = c//2, head group g = c%2 (8 heads each).
  Each core computes a partial output  z_g[b] @ Wo[g] : [2048, 1024];
  host sums the two head-group partials per batch.

Per-core kernel (all matmul inputs bf16 = full PE rate, psum fp32):
  - host passes x[b]^T (d_model on the SBUF partition dim everywhere)
  - fused single pass over 4 query chunks of 512:
      proj (Q^T,K^T,V for the chunk) -> causal attention over k-blocks of
      128.  Per k-block one merged psum tile holds BOTH heads of a pair
      (tile_position row-split pair -> the two 64-contraction matmuls
      co-issue on disjoint PE row halves); diagonal blocks get the causal
      mask folded in as a -1e9 strictly-lower-tri PE accumulate (identity
      stationary) so the single merged exp on ACT emits exact zeros;
      V-augmented-with-ones matmuls accumulate z^T and the softmax
      denominator; divide = ACT ln/exp reciprocal + co-issued K=1
      broadcast matmul pair -> output projection -> split DMA out.
  - startup is DMA-bandwidth bound: critical loads (x chunk 0, W_Q, W_K)
    are dispatched first from three engines; W_V/W_O/constants deferred.
    Chunk-0 head-pair-0 attention interleaves into the projections.
"""
import numpy as np

import concourse.bass as bass
import concourse.tile as tile
import concourse.mybir as mybir
from concourse.vector_clock import ScopedClock
from concourse.bass_utils import run_bass_kernel_spmd

D_MODEL = 1024
D_HEAD = 64
B = 4
T = 2048
H = 8              # heads per core
HG = H * D_HEAD    # 512 head-dim columns per core
TCH = 512          # q/t chunk
NCH = T // TCH     # 4
NDM = D_MODEL // 128  # 8 d_model chunks

F32R = mybir.dt.float32r
F32 = mybir.dt.float32
BF16 = mybir.dt.bfloat16
F8 = mybir.dt.float8e4
AF = mybir.ActivationFunctionType
DR = mybir.MatmulPerfMode.DoubleRow


class _TC(tile.TileContext):
    """TileContext whose tail drain carries no sem waits (this walrus build
    rejects >1 sync wait per instruction and any wait on a Drain)."""

    def _drain_and_barrier(self, tick_clock, wait_clock):
        drain_inst = self.nc.sync.drain()
        wait_clock.add_sem_waits(
            drain_inst.ins, ScopedClock({None: tick_clock.global_clock})
        )
        si = drain_inst.ins.sync_info
        waits = list(si.on_wait) if si is not None else []
        if waits:
            drain_inst.ins.sync_info = mybir.SyncInfo(
                on_wait=[], on_update=list(si.on_update)
            )
            for w in waits:
                nop = self.nc.sync.nop(nofuse=True)
                nop.ins.sync_info = mybir.SyncInfo(on_wait=[w], on_update=[])
        self.nc.all_engine_barrier()
        popped = self.nc._tile_sem_poison_stack.pop()
        assert popped is self._sem_poison
        self.nc.clear_and_free_semaphores(list(self.sems.allocated().values()))
        self.nc.all_engine_barrier()


def _split_multi_waits(nc):
    """Move all-but-one sem wait of every instruction onto same-engine NOPs."""
    cnt = 0
    for f in nc.m.functions:
        for b in f.blocks:
            new = []
            for inst in b.instructions:
                si = inst.sync_info
                if si is not None and si.on_wait is not None:
                    waits = list(si.on_wait)
                    max_keep = 0 if inst.opcode == "Drain" else 1
                    if len(waits) > max_keep:
                        keep = waits[len(waits) - max_keep:] if max_keep else []
                        spill = waits[: len(waits) - max_keep]
                        for w in spill:
                            nop = mybir.InstNoOp(
                                name=f"I-wsplit-{cnt}", engine=inst.engine,
                                ins=[], outs=[],
                            )
                            nop.sync_info = mybir.SyncInfo(
                                on_wait=[w], on_update=[]
                            )
                            new.append(nop)
                            cnt += 1
                        inst.sync_info = mybir.SyncInfo(
                            on_wait=keep, on_update=list(si.on_update)
                        )
                new.append(inst)
            b.instructions = new
    return cnt


def _build():
    nc = bass.Bass("TRN2", target_bir_lowering=False)
    xT = nc.dram_tensor("xT", (D_MODEL, T), BF16, kind="ExternalInput")
    wq = nc.dram_tensor("wq", (D_MODEL, HG), BF16, kind="ExternalInput")
    wk = nc.dram_tensor("wk", (D_MODEL, HG), BF16, kind="ExternalInput")
    wv = nc.dram_tensor("wv", (D_MODEL, HG), BF16, kind="ExternalInput")
    wo = nc.dram_tensor("wo", (HG, D_MODEL), BF16, kind="ExternalInput")
    ident = nc.dram_tensor("ident", (128, 128), BF16, kind="ExternalInput")
    ntri = nc.dram_tensor("ntri", (128, 128), BF16, kind="ExternalInput")
    ones1 = nc.dram_tensor("ones1", (33, 64), BF16, kind="ExternalInput")
    vones = nc.dram_tensor("vones", (128, T // 128, H, 1), BF16,
                           kind="ExternalInput")
    out = nc.dram_tensor("out", (T, D_MODEL), F32, kind="ExternalOutput")

    from contextlib import ExitStack
    with _TC(nc) as tc, ExitStack() as ctx:
        consts = ctx.enter_context(tc.tile_pool(name="consts", bufs=1))
        xs_pool = ctx.enter_context(tc.tile_pool(name="xs", bufs=3))
        kt_pool = ctx.enter_context(tc.tile_pool(name="kt", bufs=1))
        v_pool = ctx.enter_context(tc.tile_pool(name="v", bufs=1))
        qt_pool = ctx.enter_context(tc.tile_pool(name="qt", bufs=3))
        zt_pool = ctx.enter_context(tc.tile_pool(name="zt", bufs=4))
        et_pool = ctx.enter_context(tc.tile_pool(name="et", bufs=8))
        sm_pool = ctx.enter_context(tc.tile_pool(name="sm", bufs=4))
        ou_pool = ctx.enter_context(tc.tile_pool(name="ou", bufs=3))
        ps_s = ctx.enter_context(tc.tile_pool(name="ps_s", bufs=2, space="PSUM"))
        ps_u = ctx.enter_context(tc.tile_pool(name="ps_u", bufs=2, space="PSUM"))
        ps_w = ctx.enter_context(tc.tile_pool(name="ps_w", bufs=2, space="PSUM"))

        xT_r = xT.ap().rearrange("(c p) t -> p c t", p=128)

        # resident weights / constants.  Startup is DMA-bandwidth bound:
        # per-chunk DMAs (each sprays its descriptors over all 16 engines)
        # dispatched from three engines in parallel, critical data first
        # (xs0+wq+wk feed the first projections); wv/wo/consts are deferred
        # so they don't steal bandwidth from the critical path.
        wq_sb = consts.tile([128, NDM, HG], BF16)
        xs0 = xs_pool.tile([128, NDM, TCH], BF16, name="xs", tag="xs")
        wk_sb = consts.tile([128, NDM, HG], BF16)
        wv_sb = consts.tile([128, NDM, HG], BF16)
        wq_r = wq.ap().rearrange("(c p) n -> p c n", p=128)
        wk_r = wk.ap().rearrange("(c p) n -> p c n", p=128)
        wv_r = wv.ap().rearrange("(c p) n -> p c n", p=128)
        # all critical dispatches on gpsimd: its DMA dispatch is ~25ns per
        # instruction vs ~0.6us on sync/scalar, so every transfer is in
        # flight within ~1.5us instead of staggered over ~4us.
        for c0, cw in [(0, 1), (1, 1), (2, 2), (4, 2), (6, 2)]:
            nc.gpsimd.dma_start(out=xs0[:, c0:c0 + cw, :],
                                in_=xT_r[:, c0:c0 + cw, 0:TCH])
            nc.gpsimd.dma_start(out=wq_sb[:, c0:c0 + cw, :],
                                in_=wq_r[:, c0:c0 + cw, :])
            nc.gpsimd.dma_start(out=wk_sb[:, c0:c0 + cw, :],
                                in_=wk_r[:, c0:c0 + cw, :])
        for c in range(0, NDM, 2):
            nc.scalar.dma_start(out=wv_sb[:, c:c + 2, :], in_=wv_r[:, c:c + 2, :])
        id_sb = consts.tile([128, 128], BF16)
        ntri_sb = consts.tile([128, 128], BF16)
        ones_sb = consts.tile([33, 64], BF16)
        wo_sb = consts.tile([128, HG // 128, D_MODEL], BF16)
        # per-chunk K^T tiles [pair-packed 128, pair, t-in-chunk] and V tiles
        # (V has a ones column so row 64 of U accumulates the denominator)
        kt_tiles = [kt_pool.tile([128, 4, TCH], BF16, name=f"kt{i}", tag=f"kt{i}")
                    for i in range(NCH)]
        v_tiles = [v_pool.tile([128, 4, H, D_HEAD + 1], BF16, name=f"v{i}",
                               tag=f"v{i}") for i in range(NCH)]
        vo_r = vones.ap().rearrange("p (a b) h o -> p a b h o", b=4)

        def late_dmas():
            nc.gpsimd.dma_start(out=id_sb, in_=ident.ap())
            nc.gpsimd.dma_start(out=ntri_sb, in_=ntri.ap())
            nc.gpsimd.dma_start(out=ones_sb, in_=ones1.ap())
            nc.scalar.dma_start(
                out=wo_sb, in_=wo.ap().rearrange("(c p) n -> p c n", p=128))
            for i in range(NCH):
                nc.gpsimd.dma_start(out=v_tiles[i][:, :, :, D_HEAD:],
                                    in_=vo_r[:, i])
        late_dmas()
        # dependency-free dummy activation: hoists the ~1.3us
        # ACT_TABLE_LOAD off the first real exp's critical path (junk in,
        # junk out; the Exp/Ln table is shared).
        atl_a = sm_pool.tile([1, 128], F32, name="atl_a")
        atl_b = sm_pool.tile([1, 128], F32, name="atl_b")
        nc.vector.memset(atl_a, 0.0)
        nc.scalar.activation(out=atl_b, in_=atl_a, func=AF.Exp, scale=0.125)

        def proj_units(ch, xs, qt_sb):
            units = []
            for dqc in range(4):
                def uq(dqc=dqc):
                    pq = ps_w.tile([128, TCH], F32, tag="ps_w", name="pq")
                    for c in range(NDM):
                        nc.tensor.matmul(
                            pq, lhsT=wq_sb[:, c, dqc * 128:(dqc + 1) * 128],
                            rhs=xs[:, c, :], start=(c == 0), stop=(c == NDM - 1))
                    nc.vector.tensor_copy(out=qt_sb[:, dqc, :], in_=pq)
                units.append(uq)
                def uk(dqc=dqc):
                    pk = ps_w.tile([128, TCH], F32, tag="ps_w", name="pk")
                    for c in range(NDM):
                        nc.tensor.matmul(
                            pk, lhsT=wk_sb[:, c, dqc * 128:(dqc + 1) * 128],
                            rhs=xs[:, c, :], start=(c == 0), stop=(c == NDM - 1))
                    nc.vector.tensor_copy(out=kt_tiles[ch][:, dqc, :], in_=pk)
                units.append(uk)
            for tt in range(4):
                def uv(tt=tt):
                    pv = ps_w.tile([128, HG], F32, tag="ps_w", name="pv")
                    for c in range(NDM):
                        nc.tensor.matmul(
                            pv, lhsT=xs[:, c, tt * 128:(tt + 1) * 128],
                            rhs=wv_sb[:, c, :], start=(c == 0), stop=(c == NDM - 1))
                    nc.vector.tensor_copy(
                        out=v_tiles[ch][:, tt, :, 0:D_HEAD],
                        in_=pv.rearrange("p (h d) -> p h d", h=H))
                units.append(uv)
            return units

        def outproj_units(ch, zt_sb):
            units = []
            q0 = ch * TCH
            for tt in range(4):
                def uo(tt=tt):
                    o_sb = ou_pool.tile([128, D_MODEL], F32, name="o_sb")
                    r0 = q0 + tt * 128
                    for dc in range(2):
                        po = ps_w.tile([128, 512], F32, tag="ps_w", name="po")
                        for kc in range(4):
                            nc.tensor.matmul(
                                po, lhsT=zt_sb[:, kc, tt * 128:(tt + 1) * 128],
                                rhs=wo_sb[:, kc, dc * 512:(dc + 1) * 512],
                                start=(kc == 0), stop=(kc == 3))
                        nc.vector.tensor_copy(
                            out=o_sb[:, dc * 512:(dc + 1) * 512], in_=po)
                        nc.sync.dma_start(
                            out=out.ap()[r0:r0 + 128, dc * 512:(dc + 1) * 512],
                            in_=o_sb[:, dc * 512:(dc + 1) * 512])
                units.append(uo)
            return units

        def attention_units(ch, qt_sb, zt_sb):
            """Units: per head-pair, per k-block: a score pair (one merged
            psum tile, tile_position row-split -> co-issue), causal mask
            folded in as a -1e9 PE accumulate, one merged exp straight to
            fp8; per k-block PAIR one fp8 DoubleRow AV per head (contracts
            both k-blocks at 2 values/partition = half the PE rows); then
            the division."""
            units = []
            nkb = 4 * ch + 4
            state = {}
            for hp in range(4):
                def u_alloc(hp=hp):
                    state[hp] = [ps_u.tile([D_HEAD + 1, TCH], F32, name="u_ps",
                                           tag="u_ps") for _ in range(2)]
                units.append(u_alloc)
                for kb in range(nkb):
                    def u_kb(hp=hp, kb=kb):
                        u_ps = state[hp]
                        j = kb - 4 * ch
                        ca = 128 * j if j > 0 else 0
                        kt_t = kt_tiles[kb // 4]
                        off = (kb % 4) * 128
                        sp = ps_s.tile([128, 2, TCH], F32, name="sp", tag="s2")
                        diag = j >= 0
                        for par in range(2):
                            p0 = 64 * par
                            nc.tensor.matmul(
                                sp[:, par, ca:],
                                lhsT=kt_t[p0:p0 + 64, hp, off:off + 128],
                                rhs=qt_sb[p0:p0 + 64, hp, ca:],
                                start=True, stop=not diag,
                                tile_position=(p0, 0))
                        if diag:
                            for par in range(2):
                                nc.tensor.matmul(
                                    sp[:, par, ca:ca + 128],
                                    lhsT=id_sb, rhs=ntri_sb,
                                    start=False, stop=True)
                        et = et_pool.tile([128, 2, TCH], BF16, name="et",
                                          tag="et")
                        spf = sp.rearrange("p a b -> p (a b)")
                        etf = et.rearrange("p a b -> p (a b)")
                        if j >= 2:
                            # deep diagonal: the merged range would exp the
                            # par1 garbage gap [TCH:TCH+ca); two exps are
                            # cheaper and release par0's AV earlier.
                            nc.scalar.activation(
                                out=etf[:, ca:TCH], in_=spf[:, ca:TCH],
                                func=AF.Exp, scale=0.125)
                            nc.scalar.activation(
                                out=etf[:, TCH + ca:], in_=spf[:, TCH + ca:],
                                func=AF.Exp, scale=0.125)
                        else:
                            nc.scalar.activation(
                                out=etf[:, ca:], in_=spf[:, ca:],
                                func=AF.Exp, scale=0.125)
                        for par in range(2):
                            h = 2 * hp + par
                            nc.tensor.matmul(
                                u_ps[par][:, ca:],
                                lhsT=v_tiles[kb // 4][:, kb % 4, h, :],
                                rhs=et[:, par, ca:],
                                start=(kb == 0), stop=(kb == nkb - 1))
                    units.append(u_kb)
                def u_div(hp=hp):
                    # zt = U[0:64] / D (D = U row 64): 1/D via DVE
                    # reciprocal; broadcast over 64 partitions with a pair of
                    # K=1 matmuls co-issued at PE row positions 0 and 32
                    # (rcp rows live at partitions 0 and 32 to feed them).
                    u_ps = state[hp]
                    # 1/D = exp(-ln D) on ACT (DVE reciprocal is iterative
                    # ~3.4us; custom DVE approx ops fail codegen; ACT
                    # Reciprocal swaps tables).  Both pars' ln land in one
                    # tile (rows 0/32) and a SINGLE exp covers both, so the
                    # two K=1 broadcast matmuls share one release event and
                    # co-issue at PE row positions 0/32.  Rows 1-31 hold
                    # junk; only rows 0/32 are ever read downstream.
                    lnd = sm_pool.tile([33, TCH], F32, name="lnd")
                    for par in range(2):
                        nc.scalar.activation(
                            out=lnd[32 * par:32 * par + 1, :],
                            in_=u_ps[par][D_HEAD:D_HEAD + 1, :],
                            func=AF.Ln)
                    rcp = sm_pool.tile([33, TCH], BF16, name="rcp")
                    nc.scalar.activation(out=rcp, in_=lnd,
                                         func=AF.Exp, scale=-1.0)
                    db_ps = ps_s.tile([64, 2, TCH], F32, tag="s2", name="db_ps")
                    for par in range(2):
                        nc.tensor.matmul(
                            db_ps[:, par, :],
                            lhsT=ones_sb[32 * par:32 * par + 1, :],
                            rhs=rcp[32 * par:32 * par + 1, :],
                            start=True, stop=True,
                            tile_position=(32 * par, 0))
                    rb = sm_pool.tile([64, 2, TCH], F32, name="rb")
                    nc.vector.tensor_copy(out=rb, in_=db_ps)
                    for par in range(2):
                        nc.vector.tensor_mul(
                            zt_sb[64 * par:64 * par + 64, hp, :],
                            u_ps[par][0:D_HEAD, :], rb[:, par, :])
                u_div.is_div = True
                units.append(u_div)
            return units

        # ---- software-pipelined emission ----
        # chunk 0 projections up front; then for each chunk, its attention
        # units interleaved with (prev chunk's out-proj + next chunk's proj).
        qt_tiles = [None] * NCH
        xs_tiles = [xs0] + [None] * (NCH - 1)
        zt_tiles = [None] * NCH

        def stage_proj(ch):
            if ch >= NCH:
                return []
            if ch > 0:
                xs_tiles[ch] = xs_pool.tile([128, NDM, TCH], BF16, name="xs",
                                            tag="xs")
                nc.sync.dma_start(
                    out=xs_tiles[ch],
                    in_=xT_r[:, :, ch * TCH:(ch + 1) * TCH])
            qt_tiles[ch] = qt_pool.tile([128, 4, TCH], BF16, name="qt",
                                        tag="qt")
            return proj_units(ch, xs_tiles[ch], qt_tiles[ch])

        # chunk-0 prologue: interleave head-pair 0's attention into proj0 as
        # its inputs become ready (uq0/uk_j/uv_j), so ACT exps start early
        # and the DMA-paced startup has PE work to chew on.
        P0 = stage_proj(0)  # [uq0,uk0, uq1,uk1, uq2,uk2, uq3,uk3, uv0..uv3]
        zt_tiles[0] = zt_pool.tile([128, 4, TCH], BF16, name="zt", tag="zt")
        A0 = attention_units(0, qt_tiles[0], zt_tiles[0])
        for u in [P0[0], P0[1], P0[8], A0[0], A0[1],
                  P0[3], P0[9], A0[2],
                  P0[5], P0[10], A0[3],
                  P0[7], P0[11], A0[4], A0[5]]:
            u()
        # fill schedule: att0 | proj1, att1 | proj2, att2 | proj3+outproj0,
        # att3 | outproj1+outproj2, then outproj3.  Later chunks have more
        # attention (ACT-paced) so they get the spare PE fill.
        for ch in range(NCH):
            if ch == 0:
                au = ([P0[2]] + A0[6:12] + [P0[4]] + A0[12:18]
                      + [P0[6]] + A0[18:24])
            else:
                zt_tiles[ch] = zt_pool.tile([128, 4, TCH], BF16, name="zt",
                                            tag="zt")
                au = attention_units(ch, qt_tiles[ch], zt_tiles[ch])
            fill = []
            if ch < NCH - 1:
                fill += stage_proj(ch + 1)
            if ch == 3:
                fill += outproj_units(0, zt_tiles[0])
                fill += outproj_units(1, zt_tiles[1])
                fill += outproj_units(2, zt_tiles[2])
            k = 0
            for i, a in enumerate(au):
                if getattr(a, "is_div", False):
                    # pull fill ahead of the divide: its broadcast matmul
                    # waits ~1.3us for the ACT reciprocal and the PE issues
                    # in order, so anything behind it would stall too.
                    want = min(len(fill), k + 3)
                    while k < want:
                        fill[k]()
                        k += 1
                a()
                want = (i + 1) * len(fill) // len(au)
                while k < want:
                    fill[k]()
                    k += 1
            while k < len(fill):
                fill[k]()
                k += 1
        for u in outproj_units(NCH - 1, zt_tiles[NCH - 1]):
            u()


    _split_multi_waits(nc)
    return nc


_NC_CACHE = None


def _get_nc():
    global _NC_CACHE
    if _NC_CACHE is None:
        _NC_CACHE = _build()
    return _NC_CACHE


def _make_in_maps(x, W_Q, W_K, W_V, W_O):
    import ml_dtypes
    bf16 = ml_dtypes.bfloat16
    x = np.asarray(x, dtype=np.float32)
    W_Q = np.asarray(W_Q, dtype=np.float32).astype(bf16)
    W_K = np.asarray(W_K, dtype=np.float32).astype(bf16)
    W_V = np.asarray(W_V, dtype=np.float32).astype(bf16)
    W_O = np.asarray(W_O, dtype=np.float32).astype(bf16)

    f8 = ml_dtypes.float8_e4m3
    ident = np.eye(128, dtype=np.float32).astype(bf16)
    # mask[k, q] = -1e9 where q < k (strictly below diagonal), else 0
    ntri = np.tril(np.full((128, 128), -1e9, dtype=np.float32), k=-1).astype(bf16)
    ones1 = np.ones((33, 64), dtype=bf16)
    vones = np.ones((128, T // 128, H, 1), dtype=bf16)

    in_maps = []
    for core in range(8):
        b, g = core // 2, core % 2
        cs = slice(g * HG, (g + 1) * HG)
        in_maps.append({
            "xT": np.ascontiguousarray(x[b].T).astype(bf16),
            "wq": np.ascontiguousarray(W_Q[:, cs]),
            "wk": np.ascontiguousarray(W_K[:, cs]),
            "wv": np.ascontiguousarray(W_V[:, cs]),
            "wo": np.ascontiguousarray(W_O[cs, :]),
            "ident": ident, "ntri": ntri, "ones1": ones1, "vones": vones,
        })
    return in_maps


def kernel(x, W_Q, W_K, W_V, W_O):
    in_maps = _make_in_maps(x, W_Q, W_K, W_V, W_O)
    nc = _get_nc()
    res = run_bass_kernel_spmd(nc, in_maps, core_ids=list(range(8)))
    outs = [res.results[c]["out"] for c in range(8)]
    full = np.stack([outs[2 * b] + outs[2 * b + 1] for b in range(B)], axis=0)
    return full
